# revision 22
# baseline (speedup 1.0000x reference)
"""BiMamba (bidirectional Mamba-1 selective scan) on 8 Trainium2 NeuronCores.

Sharding: core c = (b, dir, half) with b = c>>2, dir = (c>>1)&1, half = c&1.
Each core computes its half of d_inner for one (batch, direction) in a
transposed [d, L] layout, L split in two chunks ("halves") chained through
the scan state. Program order = front(0), front(1), back(0), back(1) so
half-1's matmul-heavy front overlaps half-0's DVE-bound scan block.

front: in_proj (bf16 matmuls) -> depthwise conv (bf16 diagonal-weight
  matmuls) -> silu -> x_proj partial -> pairwise AllReduce of x_dbl [96, LC].
back: dt = softplus via exp+ln (ACT) -> per (n, d-tile):
  dA = exp(A*dt) on ACT (fp16), dBu = (dt*u)*B (DVE bf16, 2x mode),
  h = tensor_tensor_scan (DVE, fp32 internal state), hc = h*C and
  y += hc (DVE bf16) -> gate with silu(z) -> out_proj partial (bf16).
GPSIMD is kept idle: concurrent GPSIMD traffic slows DVE ~1.5x (shared
SBUF ports / activity throttle), and the scan ISA op is DVE-only.
Host sums the pair partials and concatenates directions.
"""
import sys
sys.path.insert(0, "/opt/trn_rl_repo")
import numpy as np
from contextlib import ExitStack

import concourse.bass as bass
import concourse.mybir as mybir
import concourse.tile as tile
from concourse.vector_clock import ScopedClock

F32 = mybir.dt.float32
F32R = mybir.dt.float32r
BF16 = mybir.dt.bfloat16
F16 = mybir.dt.float16
AF = mybir.ActivationFunctionType
OP = mybir.AluOpType

# ---------------------------------------------------------------- geometry
B, L, DM = 2, 2048, 1024
DI, DS, DC, DTR = 2 * DM, 16, 4, DM // 16
DH = DI // 2              # d_inner half per core
NT = DH // 128            # d-tiles per core
HALVES = 2
LC = L // HALVES          # L chunk per phase
MMT = 512                 # matmul free-dim tile

MAXW = 1                  # codegen limit: sem waits per instruction


# ------------------------------------------------------------- tile patch
def _patched_drain_and_barrier(self, tick_clock, wait_clock):
    nop_inst = self.nc.sync.nop(nofuse=True)
    wait_clock.add_sem_waits(
        nop_inst.ins, ScopedClock({None: tick_clock.global_clock}))
    si = nop_inst.ins.sync_info
    if si is not None and si.on_wait and len(si.on_wait) > MAXW:
        extra = list(si.on_wait[MAXW:])
        del si.on_wait[MAXW:]
        for i in range(0, len(extra), MAXW):
            nop2 = self.nc.sync.nop(nofuse=True)
            nop2.ins.sync_info = mybir.SyncInfo(
                on_wait=extra[i:i + MAXW], on_update=[])
    self.nc.sync.drain()
    self.nc.all_engine_barrier()
    assert self.sems is not None
    popped = self.nc._tile_sem_poison_stack.pop()
    assert popped is self._sem_poison
    self.nc.clear_and_free_semaphores(list(self.sems.allocated().values()))
    self.nc.all_engine_barrier()


tile.TileContext._drain_and_barrier = _patched_drain_and_barrier


def split_multiwaits(nc, maxw=MAXW):
    ctr = 0
    for fn in nc.m.functions:
        for blk in fn.blocks:
            il = list(blk.instructions)
            out = []
            changed = False
            for ins in il:
                si = getattr(ins, "sync_info", None)
                waits = list(si.on_wait) if (si is not None and si.on_wait) else []
                if len(waits) > maxw:
                    changed = True
                    extra, keep = waits[:-maxw], waits[-maxw:]
                    for i in range(0, len(extra), maxw):
                        nop = mybir.InstNoOp(name=f"wsplit_{ctr}", ins=[], outs=[])
                        ctr += 1
                        nop.engine = ins.engine
                        nop.sync_info = mybir.SyncInfo(
                            on_wait=extra[i:i + maxw], on_update=[])
                        out.append(nop)
                    si.on_wait = keep
                out.append(ins)
            if changed:
                blk.instructions = out
    return ctr


# ------------------------------------------------------------ bass builder
def build_nc():
    nc = bass.Bass()
    P = 128
    LTN = LC // MMT       # matmul L-tiles per half
    KT = DM // P          # d_model tiles (in_proj contraction, out rows)

    xt_d = nc.declare_dram_parameter("xt", [DM, L], BF16, isOutput=False)
    win_d = nc.declare_dram_parameter("w_in", [DM, 2 * DH], BF16,
                                      isOutput=False)
    cdiag_d = nc.declare_dram_parameter("conv_diag", [NT, DC, P, P], BF16,
                                        isOutput=False)
    cb_d = nc.declare_dram_parameter("conv_b", [P, NT], F32, isOutput=False)
    wx_d = nc.declare_dram_parameter("w_x", [DH, 96], BF16, isOutput=False)
    wdt_d = nc.declare_dram_parameter("w_dt", [DTR, DH], BF16, isOutput=False)
    dtb_d = nc.declare_dram_parameter("dt_b", [P, NT], F32, isOutput=False)
    a_d = nc.declare_dram_parameter("a_cols", [P, NT, DS], F32, isOutput=False)
    dcol_d = nc.declare_dram_parameter("d_col", [P, NT], F32, isOutput=False)
    wout_d = nc.declare_dram_parameter("w_out", [DH, DM], BF16, isOutput=False)
    zpad_d = nc.declare_dram_parameter("zpad", [P, DC - 1], BF16,
                                       isOutput=False)
    outp_d = nc.declare_dram_parameter("outp", [DM, L], F32, isOutput=True)

    ccin = [nc.dram_tensor(f"ccin{h}", [96, LC], F32) for h in range(HALVES)]
    ccout = [nc.dram_tensor(f"ccout{h}", [96, LC], F32) for h in range(HALVES)]
    bc16_d = [nc.dram_tensor(f"bc16_{h}", [2 * DS, LC], BF16)
              for h in range(HALVES)]
    groups = [[0, 1], [2, 3], [4, 5], [6, 7]]

    with tile.TileContext(nc) as tc, ExitStack() as ctx:
        pool = ctx.enter_context(tc.tile_pool(name="sb", bufs=1))
        psum = ctx.enter_context(tc.tile_pool(name="ps", bufs=6, space="PSUM"))

        # resident small weights
        wx_r = pool.tile([P, NT, 96], BF16, tag="wx")
        nc.sync.dma_start(wx_r[:], wx_d[:].rearrange("(kt p) m -> p kt m", p=P))
        wdt_r = pool.tile([DTR, NT, P], BF16, tag="wdt")
        nc.sync.dma_start(wdt_r[:], wdt_d[:].rearrange("k (mt m) -> k mt m", m=P))
        cb_sb = pool.tile([P, NT], F32, tag="cb")
        nc.sync.dma_start(cb_sb[:], cb_d[:])
        dtb_sb = pool.tile([P, NT], F32, tag="dtb")
        nc.sync.dma_start(dtb_sb[:], dtb_d[:])
        a_sb = pool.tile([P, NT, DS], F32, tag="a")
        nc.sync.dma_start(a_sb[:], a_d[:])
        dcol_sb = pool.tile([P, NT], F32, tag="dcol")
        nc.sync.dma_start(dcol_sb[:], dcol_d[:])

        halo = [pool.tile([P, DC - 1], BF16, tag=f"halo{nt}", name=f"halo{nt}")
                for nt in range(NT)]
        states = pool.tile([P, DS * NT], F32, tag="states")

        xt_re = xt_d[:].rearrange("(kt p) l -> p kt l", p=P)

        # per-half state carried from front() to back()
        u_t = [None] * HALVES
        sz_t = [None] * HALVES
        cc_ins = [None] * HALVES

        def front(half):
            l0 = half * LC
            # -------- stage 1: in_proj -> xi (bf16) and sz (bf16)
            xt_t = []
            for kt in range(KT):
                t = pool.tile([P, LC], BF16, tag="bigA", bufs=8)
                nc.sync.dma_start(t[:], xt_re[:, kt, l0:l0 + LC])
                xt_t.append(t)
            xi_t = []
            sz_t[half] = []
            for mt in range(2 * NT):
                win_t = pool.tile([P, KT, P], BF16, tag="win", bufs=3)
                nc.sync.dma_start(
                    win_t[:],
                    win_d[:, mt * P:(mt + 1) * P].rearrange(
                        "(kt p) q -> p kt q", p=P))
                if mt < NT:
                    xi = pool.tile([P, DC - 1 + LC], BF16, tag="xi", bufs=8)
                    xi_t.append(xi)
                else:
                    sz = pool.tile([P, LC], BF16, tag="sz", bufs=16)
                    sz_t[half].append(sz)
                for lt in range(LTN):
                    acc = psum.tile([P, MMT], F32, tag="mm")
                    for kt in range(KT):
                        nc.tensor.matmul(
                            acc[:], win_t[:, kt, :],
                            xt_t[kt][:, lt * MMT:(lt + 1) * MMT],
                            start=(kt == 0), stop=(kt == KT - 1))
                    if mt < NT:
                        nc.scalar.copy(
                            xi_t[mt][:, DC - 1 + lt * MMT:
                                     DC - 1 + (lt + 1) * MMT],
                            acc[:])
                    else:
                        nc.scalar.activation(
                            sz_t[half][mt - NT][:, lt * MMT:(lt + 1) * MMT],
                            acc[:], AF.Silu)

            # -------- stage 2: depthwise conv + bias + silu -> u (bf16)
            u_t[half] = []
            for nt in range(NT):
                if half == 0:
                    nc.sync.dma_start(halo[nt][:], zpad_d[:])
                nc.vector.tensor_copy(xi_t[nt][:, 0:DC - 1], halo[nt][:])
                diag_t = pool.tile([P, DC, P], BF16, tag="diag", bufs=1)
                nc.sync.dma_start(
                    diag_t[:], cdiag_d[nt].rearrange("k p q -> p k q"))
                u = pool.tile([P, LC], BF16, tag="u", bufs=16)
                for lt in range(LTN):
                    acc = psum.tile([P, MMT], F32, tag="mm")
                    for k in range(DC):
                        nc.tensor.matmul(
                            acc[:], diag_t[:, k, :],
                            xi_t[nt][:, lt * MMT + k:lt * MMT + k + MMT],
                            start=(k == 0), stop=(k == DC - 1))
                    nc.scalar.activation(
                        u[:, lt * MMT:(lt + 1) * MMT], acc[:], AF.Silu,
                        bias=cb_sb[:, nt:nt + 1])
                # save halo for next half (before xi slot recycles)
                nc.vector.tensor_copy(
                    halo[nt][:], xi_t[nt][:, LC:LC + DC - 1])
                u_t[half].append(u)

            # -------- stage 3: x_proj partial [96, LC] (PSUM -> ccin DMA)
            dma_ins = []
            for lt in range(LTN):
                acc96 = psum.tile([96, MMT], F32, tag="mm96", bufs=2)
                for nt in range(NT):
                    nc.tensor.matmul(
                        acc96[:], wx_r[:, nt, :],
                        u_t[half][nt][:, lt * MMT:(lt + 1) * MMT],
                        start=(nt == 0), stop=(nt == NT - 1))
                xp = pool.tile([96, MMT], F32, tag="xdblp", bufs=2)
                nc.scalar.copy(xp[:], acc96[:])
                dma_ins.append(nc.sync.dma_start(
                    ccin[half][:, lt * MMT:(lt + 1) * MMT], xp[:]))

            # -------- stage 4: pairwise AllReduce of x_dbl
            cc = nc.gpsimd.collective_compute(
                "AllReduce", OP.add, replica_groups=groups,
                ins=[ccin[half][:]], outs=[ccout[half][:]])
            for di in dma_ins:
                tile.add_dep_helper(cc.ins, di.ins, reason="cc after dma_in")
            cc_ins[half] = cc.ins

        def back(half):
            l0 = half * LC
            # -------- stage 5: dt = softplus(Wdt@dtr + b); dtu; y=D*u
            xdbl = pool.tile([96, LC], F32, tag="xdbl", bufs=1)
            dma_out = nc.sync.dma_start(xdbl[:], ccout[half][:])
            tile.add_dep_helper(dma_out.ins, cc_ins[half],
                                reason="read after cc")
            dtr_r = pool.tile([DTR, LC], BF16, tag="dtr", bufs=2)
            nc.scalar.copy(dtr_r[:], xdbl[0:DTR, :])
            # bf16 copy of the B/C rows for cheap broadcasts + 2x DVE ops
            xbc16 = pool.tile([2 * DS, LC], BF16, tag="xbc16", bufs=2)
            nc.vector.tensor_copy(xbc16[:], xdbl[DTR:DTR + 2 * DS, :])
            nc.sync.dma_start(bc16_d[half][:], xbc16[:])
            dt_t = []
            for nt in range(NT):
                dt = pool.tile([P, LC], BF16, tag="dt", bufs=9)
                for lt in range(LTN):
                    acc = psum.tile([P, MMT], F32, tag="mm")
                    nc.tensor.matmul(
                        acc[:], wdt_r[:, nt, :],
                        dtr_r[:, lt * MMT:(lt + 1) * MMT],
                        start=True, stop=True)
                    e = pool.tile([P, MMT], BF16, tag="spe", bufs=2)
                    nc.scalar.activation(e[:], acc[:], AF.Exp,
                                         bias=dtb_sb[:, nt:nt + 1])
                    nc.scalar.activation(
                        dt[:, lt * MMT:(lt + 1) * MMT], e[:], AF.Ln, bias=1.0)
                dt_t.append(dt)

            u16_t = []
            y16_t = []
            for nt in range(NT):
                y16 = pool.tile([P, LC], BF16, tag="y16", bufs=8)
                nc.scalar.mul(y16[:], u_t[half][nt][:],
                              dcol_sb[:, nt:nt + 1])                # y = D*u
                u16 = pool.tile([P, LC], BF16, tag="u16", bufs=8)
                nc.vector.tensor_tensor(u16[:], dt_t[nt][:],
                                        u_t[half][nt][:], OP.mult)  # dt*u
                u16_t.append(u16)
                y16_t.append(y16)

            # -------- stage 6: selective scan
            for n in range(DS):
                Bb = pool.tile([P, LC], BF16, tag="bc", bufs=3)
                nc.sync.dma_start(
                    Bb[:], bc16_d[half][n:n + 1, :].partition_broadcast(P))
                Cb = pool.tile([P, LC], BF16, tag="bc", bufs=3)
                nc.sync.dma_start(
                    Cb[:], bc16_d[half][DS + n:DS + n + 1, :]
                    .partition_broadcast(P))
                for nt in range(NT):
                    dA = pool.tile([P, LC], F16, tag="tr", bufs=3)
                    nc.scalar.activation(dA[:], dt_t[nt][:], AF.Exp,
                                         scale=a_sb[:, nt, n:n + 1])
                    dBu = pool.tile([P, LC], BF16, tag="tr16", bufs=5)
                    nc.vector.tensor_tensor(
                        dBu[:], u16_t[nt][:], Bb[:], OP.mult)
                    h = pool.tile([P, LC], BF16, tag="tr16", bufs=5)
                    init = 0.0 if half == 0 else states[:, n * NT + nt:
                                                        n * NT + nt + 1]
                    nc.vector.tensor_tensor_scan(
                        h[:], dA[:], dBu[:], init, OP.mult, OP.add)
                    if half < HALVES - 1:
                        nc.scalar.copy(
                            states[:, n * NT + nt:n * NT + nt + 1],
                            h[:, LC - 1:LC])
                    hc = pool.tile([P, LC], BF16, tag="tr16", bufs=5)
                    nc.vector.tensor_tensor(hc[:], h[:], Cb[:], OP.mult)
                    nc.vector.tensor_tensor(y16_t[nt][:], y16_t[nt][:],
                                            hc[:], OP.add)

            # -------- stage 7: gate + out_proj partial
            yg_t = []
            for nt in range(NT):
                yg = pool.tile([P, LC], BF16, tag="u", bufs=16)
                nc.vector.tensor_tensor(yg[:], y16_t[nt][:],
                                        sz_t[half][nt][:], OP.mult)
                yg_t.append(yg)
            for mt in range(KT):
                wout_t = pool.tile([P, NT, P], BF16, tag="wout", bufs=2)
                nc.sync.dma_start(
                    wout_t[:],
                    wout_d[:, mt * P:(mt + 1) * P].rearrange(
                        "(kt p) q -> p kt q", p=P))
                for lt in range(LTN):
                    acc = psum.tile([P, MMT], F32, tag="mm")
                    for kt in range(NT):
                        nc.tensor.matmul(
                            acc[:], wout_t[:, kt, :],
                            yg_t[kt][:, lt * MMT:(lt + 1) * MMT],
                            start=(kt == 0), stop=(kt == NT - 1))
                    o = pool.tile([P, MMT], F32, tag="op", bufs=2)
                    nc.scalar.copy(o[:], acc[:])
                    nc.sync.dma_start(
                        outp_d[mt * P:(mt + 1) * P,
                               l0 + lt * MMT:l0 + (lt + 1) * MMT], o[:])

        front(0)
        front(1)
        back(0)
        back(1)

    split_multiwaits(nc)
    return nc


# ------------------------------------------------------------- host side
def _prep_core_inputs(inputs, b, dir_, half):
    pre = "f_" if dir_ == 0 else "b_"
    x = np.asarray(inputs["x"][b], dtype=np.float32)          # [L, DM]
    if dir_ == 1:
        x = x[::-1]
    sl = slice(half * DH, (half + 1) * DH)

    w_in_full = np.asarray(inputs[pre + "in_proj_w"], np.float32)  # [2DI, DM]
    w_in = np.concatenate([w_in_full[sl], w_in_full[DI + half * DH:
                                                    DI + (half + 1) * DH]], 0)
    conv_w = np.asarray(inputs[pre + "conv_w"], np.float32)[sl, 0]  # [DH, DC]
    conv_b = np.asarray(inputs[pre + "conv_b"], np.float32)[sl]
    w_x = np.asarray(inputs[pre + "x_proj_w"], np.float32)[:, sl]   # [96, DH]
    w_dt = np.asarray(inputs[pre + "dt_proj_w"], np.float32)[sl]    # [DH, DTR]
    dt_b = np.asarray(inputs[pre + "dt_proj_b"], np.float32)[sl]
    A = -np.exp(np.asarray(inputs[pre + "A_log"], np.float32))[sl]  # [DH, DS]
    Dp = np.asarray(inputs[pre + "D"], np.float32)[sl]
    w_out = np.asarray(inputs[pre + "out_proj_w"], np.float32)[:, sl]  # [DM,DH]

    cdiag = np.zeros((NT, DC, 128, 128), np.float32)
    for nt in range(NT):
        for k in range(DC):
            np.fill_diagonal(cdiag[nt, k], conv_w[nt * 128:(nt + 1) * 128, k])

    import ml_dtypes
    bf16 = ml_dtypes.bfloat16
    return {
        "xt": np.ascontiguousarray(x.T).astype(bf16),
        "w_in": np.ascontiguousarray(w_in.T).astype(bf16),
        "conv_diag": cdiag.astype(bf16),
        "conv_b": np.ascontiguousarray(conv_b.reshape(NT, 128).T),
        "w_x": np.ascontiguousarray(w_x.T).astype(bf16),
        "w_dt": np.ascontiguousarray(w_dt.T).astype(bf16),
        "dt_b": np.ascontiguousarray(dt_b.reshape(NT, 128).T),
        "a_cols": np.ascontiguousarray(
            A.reshape(NT, 128, DS).transpose(1, 0, 2)),
        "d_col": np.ascontiguousarray(Dp.reshape(NT, 128).T),
        "w_out": np.ascontiguousarray(w_out.T).astype(bf16),
        "zpad": np.zeros((128, DC - 1), bf16),
    }


_CACHE = {}


def _get_nc():
    if "nc" not in _CACHE:
        _CACHE["nc"] = build_nc()
    return _CACHE["nc"]


def _make_runner():
    """Jitted 8-core PJRT runner (no donation so it can be re-invoked for
    timing). Returns (fn, in_names, out_names, out_avals)."""
    import jax
    from jax.sharding import Mesh, PartitionSpec
    from jax.experimental.shard_map import shard_map
    from concourse import bass2jax
    from concourse.bass2jax import _bass_exec_p, install_neuronx_cc_hook

    install_neuronx_cc_hook()
    nc = _get_nc()
    pname = nc.partition_id_tensor.name if nc.partition_id_tensor else None
    in_names, out_names, out_avals = [], [], []
    for alloc in nc.m.functions[0].allocations:
        if not isinstance(alloc, mybir.MemoryLocationSet):
            continue
        name = alloc.memorylocations[0].name
        if alloc.kind == "ExternalInput":
            if name != pname:
                in_names.append(name)
        elif alloc.kind == "ExternalOutput":
            out_names.append(name)
            out_avals.append(jax.core.ShapedArray(
                tuple(alloc.tensor_shape), mybir.dt.np(alloc.dtype)))
    all_names = in_names + out_names
    if pname is not None:
        all_names = all_names + [pname]

    def _body(*args):
        operands = list(args)
        if pname is not None:
            operands.append(bass2jax.partition_id_tensor())
        outs = _bass_exec_p.bind(
            *operands, out_avals=tuple(out_avals), in_names=tuple(all_names),
            out_names=tuple(out_names), lowering_input_output_aliases=(),
            sim_require_finite=False, sim_require_nnan=False, nc=nc)
        return tuple(outs)

    devices = jax.devices()[:8]
    mesh = Mesh(np.asarray(devices), ("core",))
    nin = len(in_names) + len(out_names)
    fn = jax.jit(shard_map(
        _body, mesh=mesh, in_specs=(PartitionSpec("core"),) * nin,
        out_specs=(PartitionSpec("core"),) * len(out_names), check_rep=False),
        keep_unused=True)
    return fn, in_names, out_names, out_avals


def _get_runner():
    if "runner" not in _CACHE:
        _CACHE["runner"] = _make_runner()
    return _CACHE["runner"]


def _concat_inputs(in_maps):
    import jax
    from jax.sharding import Mesh, PartitionSpec, NamedSharding
    fn, in_names, out_names, out_avals = _get_runner()
    mesh = Mesh(np.asarray(jax.devices()[:8]), ("core",))
    sh = NamedSharding(mesh, PartitionSpec("core"))
    concat = [np.concatenate([np.asarray(m[k]) for m in in_maps], axis=0)
              for k in in_names]
    zeros = [np.zeros((8 * a.shape[0], *a.shape[1:]), a.dtype)
             for a in out_avals]
    return [jax.device_put(a, sh) for a in concat + zeros]


def _run(in_maps):
    import jax
    fn, in_names, out_names, out_avals = _get_runner()
    args = _concat_inputs(in_maps)
    outs = [np.asarray(o) for o in fn(*args)]
    return [
        {k: outs[i].reshape(8, *out_avals[i].shape)[c]
         for i, k in enumerate(out_names)}
        for c in range(8)
    ]


def run_timed(in_maps, iters=5):
    import time as _t
    import jax
    fn, *_ = _get_runner()
    args = _concat_inputs(in_maps)
    jax.block_until_ready(fn(*args))
    times = []
    for _ in range(iters):
        t0 = _t.perf_counter()
        jax.block_until_ready(fn(*args))
        times.append(_t.perf_counter() - t0)
    return min(times)


def make_in_maps(inputs):
    return [
        _prep_core_inputs(inputs, c >> 2, (c >> 1) & 1, c & 1)
        for c in range(8)
    ]


def kernel(**inputs):
    in_maps = []
    for c in range(8):
        b, dir_, half = c >> 2, (c >> 1) & 1, c & 1
        in_maps.append(_prep_core_inputs(inputs, b, dir_, half))
    res = _run(in_maps)
    out = np.zeros((B, L, 2 * DM), np.float32)
    for b in range(B):
        for dir_ in range(2):
            c0 = (b << 2) | (dir_ << 1)
            part = res[c0]["outp"] + res[c0 + 1]["outp"]     # [DM, L]
            if dir_ == 1:
                part = part[:, ::-1]
            out[b, :, dir_ * DM:(dir_ + 1) * DM] = part.T
    return out

